# revision 1
# baseline (speedup 1.0000x reference)
"""DFlashAttention kernel for Trainium2, tensor-parallel across 8 NeuronCores.

Sharding: Megatron-style head parallelism. Core c owns KV head c and Q heads
4c..4c+3 (matches repeat_interleave grouping), i.e. Wq rows [512c, 512c+512),
Wk/Wv rows [128c, 128c+128), Wo columns [512c, 512c+512). Each core computes a
partial output [QL, H]; the host sums the 8 partials (row-parallel Wo).

All layouts chosen so every matmul streams N>=256 columns (full-rate fp32r):
  - activations/weights fed feature-major (host pre-transposes)
  - Q/K kept d-major [HD, seq] for scores^T = K^T(dxk-tile) @ Q(dxq)
  - softmax over the partition axis: exp on ACT (scale fused); denominator
    accumulated on PE with a stationary ones-column alongside the PV
    accumulation; normalization via reciprocal + PE broadcast
  - V kept k-major [seq, HD] (PE-transposed after d-major projection), bf16,
    P^T bf16, so PV accumulates attn^T = V^T @ P^T in PSUM over 32 k-tiles
"""

import math
from contextlib import ExitStack

import ml_dtypes
import numpy as np

import concourse.bass as bass
import concourse.bacc as bacc
import concourse.mybir as mybir
import concourse.tile as tile
from concourse.bass_utils import run_bass_kernel_spmd

F32 = mybir.dt.float32
F32R = mybir.dt.float32r
BF16 = mybir.dt.bfloat16
AF = mybir.ActivationFunctionType
ALU = mybir.AluOpType

# Full-problem dims (hardcoded per spec)
B, QL, CTX, H = 1, 2048, 2048, 4096
NH, NKV, HD = 32, 8, 128
NCORES = 8
HPC = NH // NKV  # 4 q-heads per core (one KV head per core)




def build_program(ql=QL, ctx_len=CTX, h=H, trace_sim=False, phases="ABC", body_reps=1):
    """Build the per-core Bass program (SPMD: same program, per-core shards)."""
    s = ql + ctx_len          # total kv length
    et = h // 128             # e-tiles (contraction tiles for projections)
    kt = s // 128             # k-tiles in attention
    QC = 512                  # phase A position-chunk
    nch = ql // QC            # chunks (ctx assumed == ql)
    assert ctx_len == ql, "phase A chunking assumes ctx_len == ql"
    QB = 512                  # phase B q-block
    nqb = ql // QB
    scale = 1.0 / math.sqrt(HD)
    DQ = HPC * HD             # 512: per-core q-head dim
    hot = h // 512            # output-column chunks in Wo stage

    nc = bacc.Bacc("TRN2", target_bir_lowering=False, debug=False)

    def din(name, shape, dt_=F32):
        return nc.dram_tensor(name, shape, dt_, kind="ExternalInput").ap()

    # matmul-feeding tensors are declared float32r (same bits as f32; the PE
    # rounds internally, and the BIR verifier wants the dtype consistent)
    hiddenT = din("hiddenT", [h, ql], BF16)      # hidden_states[0].T
    targetT = din("targetT", [h, ctx_len], BF16)  # target_hidden[0].T
    cosT = din("cosT", [HD, s])            # cos[0].T
    sinT = din("sinT", [HD, s])            # sign-folded sin[0].T
    wqT = din("wqT", [h, DQ], BF16)        # Wq[shard].T
    wkT = din("wkT", [h, HD], BF16)
    wvT = din("wvT", [h, HD], BF16)
    woT = din("woT", [DQ, h], F32R)        # Wo[:, shard].T
    ones_d = din("ones", [128, 128], F32R)
    onesb_d = din("ones_bf", [128, 1], BF16)
    ident_d = din("ident", [128, 128])
    out_d = nc.dram_tensor("out", [ql, h], F32, kind="ExternalOutput").ap()

    with tile.TileContext(nc, trace_sim=trace_sim) as tc, ExitStack() as ctx:
        persist = ctx.enter_context(tc.tile_pool(name="persist", bufs=1))
        ps = ctx.enter_context(
            tc.tile_pool(name="ps", bufs=8, space=bass.MemorySpace.PSUM)
        )

        qr_sb = persist.tile([128, HPC, ql], F32R, tag="qr")    # [d, h, q]
        kr_sb = persist.tile([128, s], F32R, tag="kr")          # [d, k]
        v_sb = persist.tile([128, kt, 128], BF16, tag="v")     # [k%128, ktile, d]
        ones_sb = persist.tile([128, 128], F32R, tag="ones")
        onesb_sb = persist.tile([128, 1], BF16, tag="onesb")
        ident_sb = persist.tile([128, 128], F32, tag="ident")
        nc.sync.dma_start(ones_sb[:], ones_d[:])
        nc.sync.dma_start(onesb_sb[:], onesb_d[:])
        nc.sync.dma_start(ident_sb[:], ident_d[:])

        # ---------------- Phase A: projections + RoPE + V transpose ---------
        for _rep in range(body_reps):
          with (
              tc.tile_pool(name="wpool", bufs=1) as wpool,
              tc.tile_pool(name="apool", bufs=1) as apool,
          ):
              wq_sb = wpool.tile([128, et, DQ], BF16, tag="wq")   # [e%128, etile, d]
              wk_sb = wpool.tile([128, et, HD], BF16, tag="wk")
              wv_sb = wpool.tile([128, et, HD], BF16, tag="wv")
              nc.sync.dma_start(
                  wq_sb[:], wqT.rearrange("(e p) d -> p e d", p=128)
              )
              nc.sync.dma_start(
                  wk_sb[:], wkT.rearrange("(e p) d -> p e d", p=128)
              )
              nc.sync.dma_start(
                  wv_sb[:], wvT.rearrange("(e p) d -> p e d", p=128)
              )

              def rope(ps_tile, cos_sl, sin_sl, dst):
                  # dst = ps*cos + rot_half(ps)*sin  (sin sign pre-folded)
                  raw = apool.tile([128, QC], F32, tag="rraw", bufs=3)
                  nc.scalar.copy(raw[:], ps_tile[:])
                  swp = apool.tile([128, QC], F32, tag="rswp", bufs=3)
                  nc.sync.dma_start(swp[0:64, :], raw[64:128, :])
                  nc.sync.dma_start(swp[64:128, :], raw[0:64, :])
                  t1 = apool.tile([128, QC], F32, tag="rt1", bufs=2)
                  nc.vector.tensor_tensor(t1[:], raw[:], cos_sl, ALU.mult)
                  t2 = apool.tile([128, QC], F32, tag="rt2", bufs=2)
                  nc.vector.tensor_tensor(t2[:], swp[:], sin_sl, ALU.mult)
                  nc.vector.tensor_tensor(dst, t1[:], t2[:], ALU.add)

              for c in range(nch):
                  q0 = c * QC
                  cn = apool.tile([128, QC], F32, tag="cn", bufs=1)
                  sn = apool.tile([128, QC], F32, tag="sn", bufs=1)
                  cc = apool.tile([128, QC], F32, tag="cc", bufs=1)
                  sc = apool.tile([128, QC], F32, tag="sc", bufs=1)
                  nc.sync.dma_start(cn[:], cosT[:, ctx_len + q0:ctx_len + q0 + QC])
                  nc.sync.dma_start(sn[:], sinT[:, ctx_len + q0:ctx_len + q0 + QC])
                  nc.sync.dma_start(cc[:], cosT[:, q0:q0 + QC])
                  nc.sync.dma_start(sc[:], sinT[:, q0:q0 + QC])

                  psq = [
                      ps.tile([128, QC], F32, tag="ps", name=f"psq{i}")
                      for i in range(HPC)
                  ]
                  pskn = ps.tile([128, QC], F32, tag="ps")
                  pskc = ps.tile([128, QC], F32, tag="ps")
                  psvn = ps.tile([128, QC], F32, tag="ps")
                  psvc = ps.tile([128, QC], F32, tag="ps")

                  for e in range(et):
                      hs = apool.tile([128, QC], BF16, tag="hs", bufs=6)
                      nc.sync.dma_start(
                          hs[:], hiddenT[e * 128:e * 128 + 128, q0:q0 + QC]
                      )
                      ts_ = apool.tile([128, QC], BF16, tag="ts", bufs=6)
                      nc.sync.dma_start(
                          ts_[:], targetT[e * 128:e * 128 + 128, q0:q0 + QC]
                      )
                      st = dict(start=(e == 0), stop=(e == et - 1))
                      for hh in range(HPC):
                          nc.tensor.matmul(
                              psq[hh][:],
                              wq_sb[:, e, hh * 128:hh * 128 + 128],
                              hs[:],
                              **st,
                          )
                      nc.tensor.matmul(
                          pskn[:], wk_sb[:, e, :], hs[:], **st
                      )
                      nc.tensor.matmul(
                          psvn[:], wv_sb[:, e, :], hs[:], **st
                      )
                      nc.tensor.matmul(
                          pskc[:], wk_sb[:, e, :], ts_[:], **st
                      )
                      nc.tensor.matmul(
                          psvc[:], wv_sb[:, e, :], ts_[:], **st
                      )

                  # RoPE: Q and K_noise at positions ctx+q0.., K_ctx at q0..
                  for hh in range(HPC):
                      rope(psq[hh], cn[:], sn[:], qr_sb[:, hh, q0:q0 + QC])
                  rope(pskn, cn[:], sn[:], kr_sb[:, ctx_len + q0:ctx_len + q0 + QC])
                  rope(pskc, cc[:], sc[:], kr_sb[:, q0:q0 + QC])

                  # V: d-major [d, k] chunks -> PE transpose -> k-major bf16
                  for src, kbase in ((psvc, q0), (psvn, ctx_len + q0)):
                      vd = apool.tile([128, QC], F32, tag="vd", bufs=2)
                      nc.scalar.copy(vd[:], src[:])
                      for i in range(QC // 128):
                          pst = ps.tile([128, 128], F32, tag="ps")
                          nc.tensor.transpose(
                              pst[:], vd[:, i * 128:i * 128 + 128], ident_sb[:]
                          )
                          j = (kbase + i * 128) // 128
                          nc.scalar.copy(v_sb[:, j, :], pst[:])

          # ---------------- Phase B/C: attention + output projection ----------
          with tc.tile_pool(name="bpool", bufs=1) as bpool:
              wo_sb = bpool.tile([128, HPC, h], F32R, tag="wo")
              nc.sync.dma_start(
                  wo_sb[:], woT.rearrange("(t p) o -> p t o", p=128)
              )

              for qb in range(nqb if "B" in phases else 0):
                  qs0 = qb * QB
                  ats = []
                  for hh in range(HPC):
                      expst = bpool.tile([128, kt, QB], BF16, tag="expst", bufs=1)
                      psat = ps.tile([128, QB], F32, tag="ps")
                      psrs = ps.tile([1, QB], F32, tag="ps")
                      # software-pipelined: scores stay LOOKAHEAD tiles ahead
                      # of the PV/rowsum consumers so the PE never stalls on
                      # the ACT exp latency
                      LOOKAHEAD = 4

                      def emit_scores(j):
                          pss = ps.tile([128, QB], F32, tag="ps",
                                        name=f"pss{j}")
                          nc.tensor.matmul(
                              pss[:],
                              kr_sb[:, j * 128:j * 128 + 128],
                              qr_sb[:, hh, qs0:qs0 + QB],
                              start=True,
                              stop=True,
                          )
                          nc.scalar.activation(
                              expst[:, j, :], pss[:], AF.Exp, scale=scale
                          )

                      for j in range(min(LOOKAHEAD, kt)):
                          emit_scores(j)
                      for j in range(kt):
                          if j + LOOKAHEAD < kt:
                              emit_scores(j + LOOKAHEAD)
                          nc.tensor.matmul(
                              psat[:],
                              v_sb[:, j, :],
                              expst[:, j, :],
                              start=(j == 0),
                              stop=(j == kt - 1),
                          )
                          nc.tensor.matmul(
                              psrs[:], onesb_sb[:], expst[:, j, :],
                              start=(j == 0), stop=(j == kt - 1),
                          )
                      recip = bpool.tile([1, QB], F32R, tag="recip", bufs=2)
                      with nc.allow_low_precision(
                          reason="f32r reciprocal feeds the PE broadcast matmul"
                      ):
                          nc.vector.reciprocal(recip[:], psrs[:])
                      psb = ps.tile([128, QB], F32, tag="ps")
                      nc.tensor.matmul(
                          psb[:], ones_sb[0:1, :], recip[:],
                          start=True, stop=True,
                      )
                      at_raw = bpool.tile([128, QB], F32, tag="atraw", bufs=2)
                      nc.scalar.copy(at_raw[:], psat[:])
                      at_sb = bpool.tile([128, QB], F32R, tag="attnT", bufs=4)
                      nc.vector.tensor_tensor(at_sb[:], at_raw[:], psb[:], ALU.mult)
                      ats.append(at_sb)

                  # Wo: out[q, ho] += attnT[t][:, q-tile].T @ woT[t][:, ho-chunk]
                  for qs in range(QB // 128 if "C" in phases else 0):
                      for oc in range(hot):
                          pso = ps.tile([128, 512], F32, tag="ps")
                          for t in range(HPC):
                              nc.tensor.matmul(
                                  pso[:],
                                  ats[t][:, qs * 128:qs * 128 + 128],
                                  wo_sb[:, t, oc * 512:oc * 512 + 512],
                                  start=(t == 0),
                                  stop=(t == HPC - 1),
                              )
                          ob = bpool.tile([128, 512], F32, tag="ob", bufs=2)
                          if (qs + oc) % 2 == 0:
                              nc.scalar.copy(ob[:], pso[:])
                          else:
                              nc.vector.tensor_copy(ob[:], pso[:])
                          nc.sync.dma_start(
                              out_d[qs0 + qs * 128:qs0 + qs * 128 + 128,
                                    oc * 512:oc * 512 + 512],
                              ob[:],
                          )
    return _finish(nc)


def _finish(nc):
    nc.compile()
    return nc


def make_in_maps(hidden_states, target_hidden, cos, sin, Wq, Wk, Wv, Wo):
    hidden_states = np.asarray(hidden_states, dtype=np.float32)
    target_hidden = np.asarray(target_hidden, dtype=np.float32)
    cos = np.asarray(cos, dtype=np.float32)
    sin = np.asarray(sin, dtype=np.float32)
    Wq = np.asarray(Wq, dtype=np.float32)
    Wk = np.asarray(Wk, dtype=np.float32)
    Wv = np.asarray(Wv, dtype=np.float32)
    Wo = np.asarray(Wo, dtype=np.float32)

    bf16 = ml_dtypes.bfloat16
    hT = np.ascontiguousarray(hidden_states[0].T).astype(bf16)
    tT = np.ascontiguousarray(target_hidden[0].T).astype(bf16)
    cT = np.ascontiguousarray(cos[0].T)
    sT = np.ascontiguousarray(sin[0].T).copy()
    sT[:64, :] *= -1.0  # fold rotate_half sign: rot(x)*sin == swap(x)*sT
    ident = np.eye(128, dtype=np.float32)
    ones = np.ones((128, 128), dtype=np.float32)

    in_maps = []
    for c in range(NCORES):
        in_maps.append({
            "hiddenT": hT,
            "targetT": tT,
            "cosT": cT,
            "sinT": sT,
            "wqT": np.ascontiguousarray(Wq[512 * c:512 * c + 512, :].T).astype(bf16),
            "wkT": np.ascontiguousarray(Wk[128 * c:128 * c + 128, :].T).astype(bf16),
            "wvT": np.ascontiguousarray(Wv[128 * c:128 * c + 128, :].T).astype(bf16),
            "woT": np.ascontiguousarray(Wo[:, 512 * c:512 * c + 512].T),
            "ones": ones,
            "ones_bf": np.ones((128, 1), dtype=bf16),
            "ident": ident,
        })
    return in_maps


_CACHE = {}
LAST_EXEC_NS = None
TRACE = False


def kernel(hidden_states, target_hidden, cos, sin, Wq, Wk, Wv, Wo):
    global LAST_EXEC_NS
    if "nc" not in _CACHE:
        _CACHE["nc"] = build_program()
    nc = _CACHE["nc"]
    in_maps = make_in_maps(
        hidden_states, target_hidden, cos, sin, Wq, Wk, Wv, Wo
    )
    res = run_bass_kernel_spmd(
        nc, in_maps, list(range(NCORES)), trace=TRACE
    )
    LAST_EXEC_NS = res.exec_time_ns
    out = np.zeros((QL, H), dtype=np.float32)
    for r in res.results:
        out += r["out"]
    return out.reshape(1, QL, H)



# revision 3
# speedup vs baseline: 1.2719x; 1.2719x over previous
"""DFlashAttention kernel for Trainium2, tensor-parallel across 8 NeuronCores.

Sharding: Megatron-style head parallelism. Core c owns KV head c and Q heads
4c..4c+3 (matches repeat_interleave grouping), i.e. Wq rows [512c, 512c+512),
Wk/Wv rows [128c, 128c+128), Wo columns [512c, 512c+512). Each core computes a
partial output [QL, H]; the host sums the 8 partials (row-parallel Wo).

v2 layout/pipeline notes:
  - activations/weights fed feature-major (host pre-transposes), bf16
  - Q/K kept d-major [HD, seq] f32r for scores^T = K^T(dxk-tile) @ Q(dxq)
  - scores matmul pairs write a 2-bank [128,2,512] PSUM tile; one ACT exp
    covers both halves (halves ACT instruction overhead)
  - softmax denominator: rowsum matmuls packed 4-wide into PE column groups
    via tile_position (4 concurrent M=1 matmuls ~ cost of one), partials at
    partitions 0/32/64/96 summed by a ones4 matmul; reciprocal on DVE;
    denominator broadcast along partitions via PE ones matmul
  - V kept k-major [seq, HD] (PE-transposed after d-major projection), bf16;
    PV accumulates attn^T = V^T @ P^T in PSUM over 32 k-tiles
  - Wo bf16, preloaded at kernel start; Wo matmuls of q-block n interleaved
    into the attention of q-block n+1 so the PE stays fed while ACT does exp
  - RoPE rotate-half swap via DVE partition-offset copies (no DMA);
    target_hidden loads issued on the ACT DMA queue to halve SP pressure
"""

import math
from contextlib import ExitStack

import ml_dtypes
import numpy as np

import concourse.bass as bass
import concourse.bacc as bacc
import concourse.mybir as mybir
import concourse.tile as tile
from concourse.bass_utils import run_bass_kernel_spmd

F32 = mybir.dt.float32
F32R = mybir.dt.float32r
BF16 = mybir.dt.bfloat16
AF = mybir.ActivationFunctionType
ALU = mybir.AluOpType

# Full-problem dims (hardcoded per spec)
B, QL, CTX, H = 1, 2048, 2048, 4096
NH, NKV, HD = 32, 8, 128
NCORES = 8
HPC = NH // NKV  # 4 q-heads per core (one KV head per core)


def build_program(ql=QL, ctx_len=CTX, h=H, trace_sim=False, phases="ABC", body_reps=1):
    """Build the per-core Bass program (SPMD: same program, per-core shards)."""
    s = ql + ctx_len          # total kv length
    et = h // 128             # e-tiles (contraction tiles for projections)
    kt = s // 128             # k-tiles in attention
    QC = 512                  # phase A position-chunk
    nch = ql // QC            # chunks (ctx assumed == ql)
    assert ctx_len == ql, "phase A chunking assumes ctx_len == ql"
    QB = 512                  # phase B q-block
    nqb = ql // QB
    scale = 1.0 / math.sqrt(HD)
    DQ = HPC * HD             # 512: per-core q-head dim
    hot = h // 512            # output-column chunks in Wo stage

    nc = bacc.Bacc("TRN2", target_bir_lowering=False, debug=False)

    def din(name, shape, dt_=F32):
        return nc.dram_tensor(name, shape, dt_, kind="ExternalInput").ap()

    hiddenT = din("hiddenT", [h, ql], BF16)      # hidden_states[0].T
    targetT = din("targetT", [h, ctx_len], BF16)  # target_hidden[0].T
    cosT = din("cosT", [HD, s])            # cos[0].T
    sinT = din("sinT", [HD, s])            # sign-folded sin[0].T
    wqT = din("wqT", [h, DQ], BF16)        # Wq[shard].T
    wkT = din("wkT", [h, HD], BF16)
    wvT = din("wvT", [h, HD], BF16)
    woT = din("woT", [DQ, h], BF16)        # Wo[:, shard].T
    ones_d = din("ones", [128, 128], F32R)
    onesb_d = din("ones_bf", [128, 1], BF16)
    ones4_d = din("ones4", [128, 1], F32R)  # 1.0 at partitions 0/32/64/96
    ident_d = din("ident", [128, 128])
    out_d = nc.dram_tensor("out", [ql, h], F32, kind="ExternalOutput").ap()

    with tile.TileContext(nc, trace_sim=trace_sim) as tc, ExitStack() as ctx:
        persist = ctx.enter_context(tc.tile_pool(name="persist", bufs=1))

        qr_sb = persist.tile([128, HPC, ql], F32R, tag="qr")    # [d, h, q]
        kr_sb = persist.tile([128, s], F32R, tag="kr")          # [d, k]
        v_sb = persist.tile([128, kt, 128], BF16, tag="v")     # [k%128, ktile, d]
        wo_sb = persist.tile([128, HPC, h], BF16, tag="wo")    # [t%128, t//128, o]
        ones_sb = persist.tile([128, 128], F32R, tag="ones")
        onesb_sb = persist.tile([128, 1], BF16, tag="onesb")
        ones4_sb = persist.tile([128, 1], F32R, tag="ones4")
        ident_sb = persist.tile([128, 128], F32, tag="ident")
        nc.sync.dma_start(ones_sb[:], ones_d[:])
        nc.sync.dma_start(onesb_sb[:], onesb_d[:])
        nc.sync.dma_start(ones4_sb[:], ones4_d[:])
        nc.sync.dma_start(ident_sb[:], ident_d[:])

        # ---------------- Phase A: projections + RoPE + V transpose ---------
        for _rep in range(body_reps):
          with (
              tc.tile_pool(name="wpool", bufs=1) as wpool,
              tc.tile_pool(name="apool", bufs=1) as apool,
              tc.tile_pool(name="psA", bufs=8, space=bass.MemorySpace.PSUM) as psA,
          ):
              wq_sb = wpool.tile([128, et, DQ], BF16, tag="wq")   # [e%128, etile, d]
              wk_sb = wpool.tile([128, et, HD], BF16, tag="wk")
              wv_sb = wpool.tile([128, et, HD], BF16, tag="wv")
              wq_r = wqT.rearrange("(e p) d -> p e d", p=128)
              wk_r = wkT.rearrange("(e p) d -> p e d", p=128)
              wv_r = wvT.rearrange("(e p) d -> p e d", p=128)
              wo_r = woT.rearrange("(t p) o -> p t o", p=128)

              def emit_weight_chunk(wc):
                  # interleaved into chunk 0's e-loop: first matmuls start early
                  sl = slice(wc * (et // 4), (wc + 1) * (et // 4))
                  nc.sync.dma_start(wk_sb[:, sl, :], wk_r[:, sl, :])
                  nc.sync.dma_start(wv_sb[:, sl, :], wv_r[:, sl, :])
                  nc.sync.dma_start(wq_sb[:, sl, :], wq_r[:, sl, :])

              def rope(ps_tile, cos_sl, sin_sl, dst, dve_raw=False):
                  # dst = ps*cos + rot_half(ps)*sin  (sin sign pre-folded)
                  raw = apool.tile([128, QC], F32, tag="rraw", bufs=3)
                  if dve_raw:
                      nc.vector.tensor_copy(raw[:], ps_tile[:])
                  else:
                      nc.scalar.copy(raw[:], ps_tile[:])
                  swp = apool.tile([128, QC], F32, tag="rswp", bufs=3)
                  nc.vector.tensor_copy(swp[0:64, :], raw[64:128, :])
                  nc.vector.tensor_copy(swp[64:128, :], raw[0:64, :])
                  t1 = apool.tile([128, QC], F32, tag="rt1", bufs=2)
                  nc.vector.tensor_tensor(t1[:], raw[:], cos_sl, ALU.mult)
                  t2 = apool.tile([128, QC], F32, tag="rt2", bufs=2)
                  nc.vector.tensor_tensor(t2[:], swp[:], sin_sl, ALU.mult)
                  nc.vector.tensor_tensor(dst, t1[:], t2[:], ALU.add)

              for c in range(nch):
                  q0 = c * QC
                  cn = apool.tile([128, QC], F32, tag="cn", bufs=1)
                  sn = apool.tile([128, QC], F32, tag="sn", bufs=1)
                  cc = apool.tile([128, QC], F32, tag="cc", bufs=1)
                  sc = apool.tile([128, QC], F32, tag="sc", bufs=1)
                  nc.scalar.dma_start(cn[:], cosT[:, ctx_len + q0:ctx_len + q0 + QC])
                  nc.scalar.dma_start(sn[:], sinT[:, ctx_len + q0:ctx_len + q0 + QC])
                  nc.scalar.dma_start(cc[:], cosT[:, q0:q0 + QC])
                  nc.scalar.dma_start(sc[:], sinT[:, q0:q0 + QC])

                  psq = [
                      psA.tile([128, QC], F32, tag="ps", name=f"psq{i}")
                      for i in range(HPC)
                  ]
                  pskn = psA.tile([128, QC], F32, tag="ps")
                  pskc = psA.tile([128, QC], F32, tag="ps")
                  psvn = psA.tile([128, QC], F32, tag="ps")
                  psvc = psA.tile([128, QC], F32, tag="ps")

                  for e in range(et):
                      if c == 0 and e % (et // 4) == 0:
                          emit_weight_chunk(e // (et // 4))
                      hs = apool.tile([128, QC], BF16, tag="hs", bufs=6)
                      nc.sync.dma_start(
                          hs[:], hiddenT[e * 128:e * 128 + 128, q0:q0 + QC]
                      )
                      ts_ = apool.tile([128, QC], BF16, tag="ts", bufs=6)
                      nc.scalar.dma_start(
                          ts_[:], targetT[e * 128:e * 128 + 128, q0:q0 + QC]
                      )
                      st = dict(start=(e == 0), stop=(e == et - 1))
                      for hh in range(HPC):
                          nc.tensor.matmul(
                              psq[hh][:],
                              wq_sb[:, e, hh * 128:hh * 128 + 128],
                              hs[:],
                              **st,
                          )
                      nc.tensor.matmul(pskn[:], wk_sb[:, e, :], hs[:], **st)
                      nc.tensor.matmul(psvn[:], wv_sb[:, e, :], hs[:], **st)
                      nc.tensor.matmul(pskc[:], wk_sb[:, e, :], ts_[:], **st)
                      nc.tensor.matmul(psvc[:], wv_sb[:, e, :], ts_[:], **st)

                  # RoPE + V drain. In the last chunk, finish kr/v first so
                  # phase B's first scores aren't gated on the Q ropes.
                  def drain_kv():
                      rope(pskn, cn[:], sn[:],
                           kr_sb[:, ctx_len + q0:ctx_len + q0 + QC], dve_raw=True)
                      rope(pskc, cc[:], sc[:], kr_sb[:, q0:q0 + QC])
                      for vsrc, kbase in ((psvc, q0), (psvn, ctx_len + q0)):
                          vd = apool.tile([128, QC], F32, tag="vd", bufs=2)
                          nc.scalar.copy(vd[:], vsrc[:])
                          for i in range(QC // 128):
                              pst = psA.tile([128, 128], F32, tag="ps")
                              nc.tensor.transpose(
                                  pst[:], vd[:, i * 128:i * 128 + 128], ident_sb[:]
                              )
                              j = (kbase + i * 128) // 128
                              nc.scalar.copy(v_sb[:, j, :], pst[:])

                  def drain_q():
                      for hh in range(HPC):
                          rope(psq[hh], cn[:], sn[:], qr_sb[:, hh, q0:q0 + QC],
                               dve_raw=(hh % 2 == 1))

                  if c == nch - 1:
                      drain_kv()
                      drain_q()
                  else:
                      drain_q()
                      drain_kv()

                  # wo chunk load on the ACT queue, spread across chunks
                  wsl = slice(c * (HPC // nch) if HPC >= nch else c,
                              (c + 1) * max(HPC // nch, 1))
                  nc.scalar.dma_start(wo_sb[:, wsl, :], wo_r[:, wsl, :])

          # ---------------- Phase B/C: attention + output projection ----------
          with (
              tc.tile_pool(name="bpool", bufs=1) as bpool,
              tc.tile_pool(name="psB", bufs=1, space=bass.MemorySpace.PSUM) as psB,
          ):
              ats = {}

              def emit_head(qb, hh):
                  qs0 = qb * QB
                  qsl = qr_sb[:, hh, qs0:qs0 + QB]
                  expst = bpool.tile([128, kt, QB], BF16, tag="expst", bufs=2)
                  psat = psB.tile([128, QB], F32, tag="psat", bufs=1)
                  psrs = psB.tile([128, QB], F32, tag="psrs", bufs=1)

                  def emit_scores_pair(g):
                      pss = psB.tile([128, 2, 512], F32, tag="pss", bufs=2)
                      for u in (0, 1):
                          j = 2 * g + u
                          nc.tensor.matmul(
                              pss[:, u, :],
                              kr_sb[:, j * 128:j * 128 + 128],
                              qsl,
                              start=True,
                              stop=True,
                          )
                      nc.scalar.activation(
                          expst[:, 2 * g:2 * g + 2, :], pss[:], AF.Exp, scale=scale
                      )

                  LOOK = 2
                  npair = kt // 2
                  for g in range(min(LOOK, npair)):
                      emit_scores_pair(g)
                  for g in range(npair):
                      if g + LOOK < npair:
                          emit_scores_pair(g + LOOK)
                      for u in (0, 1):
                          j = 2 * g + u
                          nc.tensor.matmul(
                              psat[:],
                              v_sb[:, j, :],
                              expst[:, j, :],
                              start=(j == 0),
                              stop=(j == kt - 1),
                          )
                      if g % 2 == 1:
                          # rowsum quad packed into 4 PE column groups
                          for r in range(4):
                              j = (g // 2) * 4 + r
                              nc.tensor.matmul(
                                  psrs[32 * r:32 * r + 1, :],
                                  onesb_sb[:],
                                  expst[:, j, :],
                                  start=(j < 4),
                                  stop=(j >= kt - 4),
                                  tile_position=(0, 32 * r),
                              )

                  # denominator: sum 4 col-group partials, reciprocal, broadcast
                  rs_sb = bpool.tile([128, QB], F32R, tag="rs", bufs=2)
                  nc.vector.tensor_copy(rs_sb[:], psrs[:])
                  psden = psB.tile([128, QB], F32, tag="psrs", bufs=1)
                  nc.tensor.matmul(
                      psden[0:1, :], ones4_sb[:], rs_sb[:], start=True, stop=True
                  )
                  recip = bpool.tile([1, QB], F32R, tag="recip", bufs=2)
                  with nc.allow_low_precision(
                      reason="f32r reciprocal feeds the PE broadcast matmul"
                  ):
                      nc.vector.reciprocal(recip[:], psden[0:1, :])
                  psb = psB.tile([128, QB], F32, tag="psb", bufs=1)
                  nc.tensor.matmul(
                      psb[:], ones_sb[0:1, :], recip[:], start=True, stop=True
                  )
                  psb_sb = bpool.tile([128, QB], F32R, tag="psbsb", bufs=2)
                  nc.vector.tensor_copy(psb_sb[:], psb[:])
                  at = bpool.tile([128, QB], BF16, tag="at", bufs=8)
                  nc.vector.tensor_tensor(at[:], psat[:], psb_sb[:], ALU.mult)
                  ats[(qb, hh)] = at

              def emit_c_chunk(qb, qs):
                  qs0 = qb * QB
                  for oc in range(hot):
                      pso = psB.tile([128, 512], F32, tag="pso", bufs=1)
                      for t in range(HPC):
                          nc.tensor.matmul(
                              pso[:],
                              ats[(qb, t)][:, qs * 128:qs * 128 + 128],
                              wo_sb[:, t, oc * 512:oc * 512 + 512],
                              start=(t == 0),
                              stop=(t == HPC - 1),
                          )
                      ob = bpool.tile([128, 512], F32, tag="ob", bufs=3)
                      if (qs + oc) % 2 == 0:
                          nc.scalar.copy(ob[:], pso[:])
                      else:
                          nc.vector.tensor_copy(ob[:], pso[:])
                      nc.sync.dma_start(
                          out_d[qs0 + qs * 128:qs0 + qs * 128 + 128,
                                oc * 512:oc * 512 + 512],
                          ob[:],
                      )

              if "B" in phases:
                  for qb in range(nqb):
                      for hh in range(HPC):
                          emit_head(qb, hh)
                          if qb > 0 and "C" in phases:
                              emit_c_chunk(qb - 1, hh)
                  if "C" in phases:
                      for qs in range(QB // 128):
                          emit_c_chunk(nqb - 1, qs)
    return _finish(nc)


def _finish(nc):
    nc.compile()
    return nc


def make_in_maps(hidden_states, target_hidden, cos, sin, Wq, Wk, Wv, Wo):
    hidden_states = np.asarray(hidden_states, dtype=np.float32)
    target_hidden = np.asarray(target_hidden, dtype=np.float32)
    cos = np.asarray(cos, dtype=np.float32)
    sin = np.asarray(sin, dtype=np.float32)
    Wq = np.asarray(Wq, dtype=np.float32)
    Wk = np.asarray(Wk, dtype=np.float32)
    Wv = np.asarray(Wv, dtype=np.float32)
    Wo = np.asarray(Wo, dtype=np.float32)

    bf16 = ml_dtypes.bfloat16
    hT = np.ascontiguousarray(hidden_states[0].T).astype(bf16)
    tT = np.ascontiguousarray(target_hidden[0].T).astype(bf16)
    cT = np.ascontiguousarray(cos[0].T)
    sT = np.ascontiguousarray(sin[0].T).copy()
    sT[:64, :] *= -1.0  # fold rotate_half sign: rot(x)*sin == swap(x)*sT
    ident = np.eye(128, dtype=np.float32)
    ones = np.ones((128, 128), dtype=np.float32)
    ones4 = np.zeros((128, 1), dtype=np.float32)
    ones4[[0, 32, 64, 96], 0] = 1.0

    in_maps = []
    for c in range(NCORES):
        in_maps.append({
            "hiddenT": hT,
            "targetT": tT,
            "cosT": cT,
            "sinT": sT,
            "wqT": np.ascontiguousarray(Wq[512 * c:512 * c + 512, :].T).astype(bf16),
            "wkT": np.ascontiguousarray(Wk[128 * c:128 * c + 128, :].T).astype(bf16),
            "wvT": np.ascontiguousarray(Wv[128 * c:128 * c + 128, :].T).astype(bf16),
            "woT": np.ascontiguousarray(Wo[:, 512 * c:512 * c + 512].T).astype(bf16),
            "ones": ones,
            "ones_bf": np.ones((128, 1), dtype=bf16),
            "ones4": ones4,
            "ident": ident,
        })
    return in_maps


_CACHE = {}
LAST_EXEC_NS = None
TRACE = False


def kernel(hidden_states, target_hidden, cos, sin, Wq, Wk, Wv, Wo):
    global LAST_EXEC_NS
    if "nc" not in _CACHE:
        _CACHE["nc"] = build_program()
    nc = _CACHE["nc"]
    in_maps = make_in_maps(
        hidden_states, target_hidden, cos, sin, Wq, Wk, Wv, Wo
    )
    res = run_bass_kernel_spmd(
        nc, in_maps, list(range(NCORES)), trace=TRACE
    )
    LAST_EXEC_NS = res.exec_time_ns
    out = np.zeros((QL, H), dtype=np.float32)
    for r in res.results:
        out += r["out"]
    return out.reshape(1, QL, H)


# revision 5
# speedup vs baseline: 1.2859x; 1.0111x over previous
"""DFlashAttention kernel for Trainium2, tensor-parallel across 8 NeuronCores.

Sharding: Megatron-style head parallelism. Core c owns KV head c and Q heads
4c..4c+3 (matches repeat_interleave grouping), i.e. Wq rows [512c, 512c+512),
Wk/Wv rows [128c, 128c+128), Wo columns [512c, 512c+512). Each core computes a
partial output [QL, H]; the host sums the 8 partials (row-parallel Wo).

v2 layout/pipeline notes:
  - activations/weights fed feature-major (host pre-transposes), bf16
  - Q/K kept d-major [HD, seq] f32r for scores^T = K^T(dxk-tile) @ Q(dxq)
  - scores matmul pairs write a 2-bank [128,2,512] PSUM tile; one ACT exp
    covers both halves (halves ACT instruction overhead)
  - softmax denominator: rowsum matmuls packed 4-wide into PE column groups
    via tile_position (4 concurrent M=1 matmuls ~ cost of one), partials at
    partitions 0/32/64/96 summed by a ones4 matmul; reciprocal on DVE;
    denominator broadcast along partitions via PE ones matmul
  - V kept k-major [seq, HD] (PE-transposed after d-major projection), bf16;
    PV accumulates attn^T = V^T @ P^T in PSUM over 32 k-tiles
  - Wo bf16, preloaded at kernel start; Wo matmuls of q-block n interleaved
    into the attention of q-block n+1 so the PE stays fed while ACT does exp
  - RoPE rotate-half swap via DVE partition-offset copies (no DMA);
    target_hidden loads issued on the ACT DMA queue to halve SP pressure
"""

import math
from contextlib import ExitStack

import ml_dtypes
import numpy as np

import concourse.bass as bass
import concourse.bacc as bacc
import concourse.mybir as mybir
import concourse.tile as tile
from concourse.bass_utils import run_bass_kernel_spmd

F32 = mybir.dt.float32
F32R = mybir.dt.float32r
BF16 = mybir.dt.bfloat16
AF = mybir.ActivationFunctionType
ALU = mybir.AluOpType

# Full-problem dims (hardcoded per spec)
B, QL, CTX, H = 1, 2048, 2048, 4096
NH, NKV, HD = 32, 8, 128
NCORES = 8
HPC = NH // NKV  # 4 q-heads per core (one KV head per core)


def build_program(ql=QL, ctx_len=CTX, h=H, trace_sim=False, phases="ABC", body_reps=1):
    """Build the per-core Bass program (SPMD: same program, per-core shards)."""
    s = ql + ctx_len          # total kv length
    et = h // 128             # e-tiles (contraction tiles for projections)
    kt = s // 128             # k-tiles in attention
    QC = 512                  # phase A position-chunk
    nch = ql // QC            # chunks (ctx assumed == ql)
    assert ctx_len == ql, "phase A chunking assumes ctx_len == ql"
    QB = 512                  # phase B q-block
    nqb = ql // QB
    scale = 1.0 / math.sqrt(HD)
    DQ = HPC * HD             # 512: per-core q-head dim
    hot = h // 512            # output-column chunks in Wo stage

    nc = bacc.Bacc("TRN2", target_bir_lowering=False, debug=False)

    def din(name, shape, dt_=F32):
        return nc.dram_tensor(name, shape, dt_, kind="ExternalInput").ap()

    hiddenT = din("hiddenT", [h, ql], BF16)      # hidden_states[0].T
    targetT = din("targetT", [h, ctx_len], BF16)  # target_hidden[0].T
    cosT = din("cosT", [HD, s])            # cos[0].T
    sinT = din("sinT", [HD, s])            # sign-folded sin[0].T
    wqT = din("wqT", [h, DQ], BF16)        # Wq[shard].T
    wkT = din("wkT", [h, HD], BF16)
    wvT = din("wvT", [h, HD], BF16)
    woT = din("woT", [DQ, h], BF16)        # Wo[:, shard].T
    onesb_d = din("ones_bf", [128, 1], BF16)
    onesr_d = din("onesr", [128, 128], F32R)  # rows 0/32/64/96 all-ones
    ident_d = din("ident", [128, 128])
    out_d = nc.dram_tensor("out", [ql, h], F32, kind="ExternalOutput").ap()

    with tile.TileContext(nc, trace_sim=trace_sim) as tc, ExitStack() as ctx:
        persist = ctx.enter_context(tc.tile_pool(name="persist", bufs=1))

        qr_sb = persist.tile([128, HPC, ql], F32R, tag="qr")    # [d, h, q]
        kr_sb = persist.tile([128, s], F32R, tag="kr")          # [d, k]
        v_sb = persist.tile([128, kt, 128], BF16, tag="v")     # [k%128, ktile, d]
        wo_sb = persist.tile([128, HPC, h], BF16, tag="wo")    # [t%128, t//128, o]
        onesb_sb = persist.tile([128, 1], BF16, tag="onesb")
        onesr_sb = persist.tile([128, 128], F32R, tag="onesr")
        ident_sb = persist.tile([128, 128], F32, tag="ident")
        nc.sync.dma_start(onesb_sb[:], onesb_d[:])
        nc.sync.dma_start(onesr_sb[:], onesr_d[:])
        nc.sync.dma_start(ident_sb[:], ident_d[:])

        # ---------------- Phase A: projections + RoPE + V transpose ---------
        for _rep in range(body_reps):
          with (
              tc.tile_pool(name="wpool", bufs=1) as wpool,
              tc.tile_pool(name="apool", bufs=1) as apool,
              tc.tile_pool(name="psA", bufs=8, space=bass.MemorySpace.PSUM) as psA,
          ):
              wq_sb = wpool.tile([128, et, DQ], BF16, tag="wq")   # [e%128, etile, d]
              wk_sb = wpool.tile([128, et, HD], BF16, tag="wk")
              wv_sb = wpool.tile([128, et, HD], BF16, tag="wv")
              wq_r = wqT.rearrange("(e p) d -> p e d", p=128)
              wk_r = wkT.rearrange("(e p) d -> p e d", p=128)
              wv_r = wvT.rearrange("(e p) d -> p e d", p=128)
              wo_r = woT.rearrange("(t p) o -> p t o", p=128)

              def emit_weight_chunk(wc):
                  # interleaved into chunk 0's e-loop: first matmuls start early
                  sl = slice(wc * (et // 4), (wc + 1) * (et // 4))
                  nc.sync.dma_start(wk_sb[:, sl, :], wk_r[:, sl, :])
                  nc.sync.dma_start(wv_sb[:, sl, :], wv_r[:, sl, :])
                  nc.sync.dma_start(wq_sb[:, sl, :], wq_r[:, sl, :])

              def rope(ps_tile, cos_sl, sin_sl, dst, dve_raw=False):
                  # dst = ps*cos + rot_half(ps)*sin  (sin sign pre-folded)
                  raw = apool.tile([128, QC], F32, tag="rraw", bufs=3)
                  if dve_raw:
                      nc.vector.tensor_copy(raw[:], ps_tile[:])
                  else:
                      nc.scalar.copy(raw[:], ps_tile[:])
                  swp = apool.tile([128, QC], F32, tag="rswp", bufs=3)
                  nc.vector.tensor_copy(swp[0:64, :], raw[64:128, :])
                  nc.vector.tensor_copy(swp[64:128, :], raw[0:64, :])
                  t1 = apool.tile([128, QC], F32, tag="rt1", bufs=2)
                  nc.vector.tensor_tensor(t1[:], raw[:], cos_sl, ALU.mult)
                  t2 = apool.tile([128, QC], F32, tag="rt2", bufs=2)
                  nc.vector.tensor_tensor(t2[:], swp[:], sin_sl, ALU.mult)
                  nc.vector.tensor_tensor(dst, t1[:], t2[:], ALU.add)

              for c in range(nch):
                  q0 = c * QC
                  cn = apool.tile([128, QC], F32, tag="cn", bufs=1)
                  sn = apool.tile([128, QC], F32, tag="sn", bufs=1)
                  cc = apool.tile([128, QC], F32, tag="cc", bufs=1)
                  sc = apool.tile([128, QC], F32, tag="sc", bufs=1)
                  nc.scalar.dma_start(cn[:], cosT[:, ctx_len + q0:ctx_len + q0 + QC])
                  nc.scalar.dma_start(sn[:], sinT[:, ctx_len + q0:ctx_len + q0 + QC])
                  nc.scalar.dma_start(cc[:], cosT[:, q0:q0 + QC])
                  nc.scalar.dma_start(sc[:], sinT[:, q0:q0 + QC])

                  psq = [
                      psA.tile([128, QC], F32, tag="ps", name=f"psq{i}")
                      for i in range(HPC)
                  ]
                  pskn = psA.tile([128, QC], F32, tag="ps")
                  pskc = psA.tile([128, QC], F32, tag="ps")
                  psvn = psA.tile([128, QC], F32, tag="ps")
                  psvc = psA.tile([128, QC], F32, tag="ps")

                  for e in range(et):
                      if c == 0 and e % (et // 4) == 0:
                          emit_weight_chunk(e // (et // 4))
                      hs = apool.tile([128, QC], BF16, tag="hs", bufs=6)
                      nc.sync.dma_start(
                          hs[:], hiddenT[e * 128:e * 128 + 128, q0:q0 + QC]
                      )
                      ts_ = apool.tile([128, QC], BF16, tag="ts", bufs=6)
                      nc.scalar.dma_start(
                          ts_[:], targetT[e * 128:e * 128 + 128, q0:q0 + QC]
                      )
                      st = dict(start=(e == 0), stop=(e == et - 1))
                      for hh in range(HPC):
                          nc.tensor.matmul(
                              psq[hh][:],
                              wq_sb[:, e, hh * 128:hh * 128 + 128],
                              hs[:],
                              **st,
                          )
                      nc.tensor.matmul(pskn[:], wk_sb[:, e, :], hs[:], **st)
                      nc.tensor.matmul(psvn[:], wv_sb[:, e, :], hs[:], **st)
                      nc.tensor.matmul(pskc[:], wk_sb[:, e, :], ts_[:], **st)
                      nc.tensor.matmul(psvc[:], wv_sb[:, e, :], ts_[:], **st)

                  # RoPE + V drain. In the last chunk, finish kr/v first so
                  # phase B's first scores aren't gated on the Q ropes.
                  def drain_kv():
                      rope(pskn, cn[:], sn[:],
                           kr_sb[:, ctx_len + q0:ctx_len + q0 + QC], dve_raw=True)
                      rope(pskc, cc[:], sc[:], kr_sb[:, q0:q0 + QC])
                      for vsrc, kbase in ((psvc, q0), (psvn, ctx_len + q0)):
                          vd = apool.tile([128, QC], F32, tag="vd", bufs=2)
                          nc.scalar.copy(vd[:], vsrc[:])
                          for i in range(QC // 128):
                              pst = psA.tile([128, 128], F32, tag="ps")
                              nc.tensor.transpose(
                                  pst[:], vd[:, i * 128:i * 128 + 128], ident_sb[:]
                              )
                              j = (kbase + i * 128) // 128
                              nc.scalar.copy(v_sb[:, j, :], pst[:])

                  def drain_q():
                      for hh in range(HPC):
                          rope(psq[hh], cn[:], sn[:], qr_sb[:, hh, q0:q0 + QC],
                               dve_raw=(hh % 2 == 1))

                  if c == nch - 1:
                      drain_kv()
                      drain_q()
                  else:
                      drain_q()
                      drain_kv()

                  # wo chunk load on the ACT queue, spread across chunks
                  wsl = slice(c * (HPC // nch) if HPC >= nch else c,
                              (c + 1) * max(HPC // nch, 1))
                  nc.scalar.dma_start(wo_sb[:, wsl, :], wo_r[:, wsl, :])

          # ---------------- Phase B/C: attention + output projection ----------
          with (
              tc.tile_pool(name="bpool", bufs=1) as bpool,
              tc.tile_pool(name="psB", bufs=1, space=bass.MemorySpace.PSUM) as psB,
          ):
              ats = {}

              def emit_head(qb, hh, between=None):
                  qs0 = qb * QB
                  qsl = qr_sb[:, hh, qs0:qs0 + QB]
                  expst = bpool.tile([128, kt, QB], BF16, tag="expst", bufs=2)
                  psat = psB.tile([128, QB], F32, tag="psat", bufs=2)
                  psrs = psB.tile([128, QB], F32, tag="psrs", bufs=1)

                  def emit_scores_pair(g):
                      pss = psB.tile([128, 2, 512], F32, tag="pss", bufs=2)
                      for u in (0, 1):
                          j = 2 * g + u
                          nc.tensor.matmul(
                              pss[:, u, :],
                              kr_sb[:, j * 128:j * 128 + 128],
                              qsl,
                              start=True,
                              stop=True,
                          )
                      nc.scalar.activation(
                          expst[:, 2 * g:2 * g + 2, :], pss[:], AF.Exp, scale=scale
                      )

                  LOOK = 2
                  npair = kt // 2
                  for g in range(min(LOOK, npair)):
                      emit_scores_pair(g)
                  for g in range(npair):
                      if g + LOOK < npair:
                          emit_scores_pair(g + LOOK)
                      for u in (0, 1):
                          j = 2 * g + u
                          nc.tensor.matmul(
                              psat[:],
                              v_sb[:, j, :],
                              expst[:, j, :],
                              start=(j == 0),
                              stop=(j == kt - 1),
                          )
                      if g % 2 == 1:
                          # rowsum quad packed into 4 PE column groups
                          for r in range(4):
                              j = (g // 2) * 4 + r
                              nc.tensor.matmul(
                                  psrs[32 * r:32 * r + 1, :],
                                  onesb_sb[:],
                                  expst[:, j, :],
                                  start=(j < 4),
                                  stop=(j >= kt - 4),
                                  tile_position=(0, 32 * r),
                              )
                      if between is not None:
                          between(g)

                  # denominator: one matmul reduces the 4 col-group partials
                  # AND broadcasts the sum to all 128 partitions (onesr has
                  # all-ones rows at partitions 0/32/64/96)
                  rs_sb = bpool.tile([128, QB], F32R, tag="rs", bufs=2)
                  nc.vector.tensor_copy(rs_sb[:], psrs[:])
                  psd = psB.tile([128, QB], F32, tag="psrs", bufs=1)
                  nc.tensor.matmul(
                      psd[:], onesr_sb[:], rs_sb[:], start=True, stop=True
                  )
                  recd = bpool.tile([128, QB], F32R, tag="recip", bufs=2)
                  with nc.allow_low_precision(
                      reason="f32r reciprocal feeds the normalize multiply"
                  ):
                      nc.vector.reciprocal(recd[:], psd[:])
                  at = bpool.tile([128, QB], BF16, tag="at", bufs=8)
                  nc.vector.tensor_tensor(at[:], psat[:], recd[:], ALU.mult)
                  ats[(qb, hh)] = at

              def emit_c_chain(qb, qs, oc):
                  qs0 = qb * QB
                  pso = psB.tile([128, 512], F32, tag="pso", bufs=1)
                  for t in range(HPC):
                      nc.tensor.matmul(
                          pso[:],
                          ats[(qb, t)][:, qs * 128:qs * 128 + 128],
                          wo_sb[:, t, oc * 512:oc * 512 + 512],
                          start=(t == 0),
                          stop=(t == HPC - 1),
                      )
                  ob = bpool.tile([128, 512], F32, tag="ob", bufs=3)
                  if (qs + oc) % 2 == 0:
                      nc.scalar.copy(ob[:], pso[:])
                  else:
                      nc.vector.tensor_copy(ob[:], pso[:])
                  nc.sync.dma_start(
                      out_d[qs0 + qs * 128:qs0 + qs * 128 + 128,
                            oc * 512:oc * 512 + 512],
                      ob[:],
                  )

              if "B" in phases:
                  for qb in range(nqb):
                      for hh in range(HPC):
                          if qb > 0 and "C" in phases:
                              # one Wo chain after every 2 score pairs: the
                              # pso-bank wait always coincides with ready
                              # attention matmuls
                              emit_head(qb, hh, between=lambda g, _q=qb - 1,
                                        _s=hh: emit_c_chain(_q, _s, g // 2)
                                        if g % 2 == 1 else None)
                          else:
                              emit_head(qb, hh)
                  if "C" in phases:
                      for qs in range(QB // 128):
                          for oc in range(hot):
                              emit_c_chain(nqb - 1, qs, oc)
    return _finish(nc)


def _finish(nc):
    nc.compile()
    return nc


def make_in_maps(hidden_states, target_hidden, cos, sin, Wq, Wk, Wv, Wo):
    hidden_states = np.asarray(hidden_states, dtype=np.float32)
    target_hidden = np.asarray(target_hidden, dtype=np.float32)
    cos = np.asarray(cos, dtype=np.float32)
    sin = np.asarray(sin, dtype=np.float32)
    Wq = np.asarray(Wq, dtype=np.float32)
    Wk = np.asarray(Wk, dtype=np.float32)
    Wv = np.asarray(Wv, dtype=np.float32)
    Wo = np.asarray(Wo, dtype=np.float32)

    bf16 = ml_dtypes.bfloat16
    hT = np.ascontiguousarray(hidden_states[0].T).astype(bf16)
    tT = np.ascontiguousarray(target_hidden[0].T).astype(bf16)
    cT = np.ascontiguousarray(cos[0].T)
    sT = np.ascontiguousarray(sin[0].T).copy()
    sT[:64, :] *= -1.0  # fold rotate_half sign: rot(x)*sin == swap(x)*sT
    ident = np.eye(128, dtype=np.float32)
    onesr = np.zeros((128, 128), dtype=np.float32)
    onesr[[0, 32, 64, 96], :] = 1.0

    in_maps = []
    for c in range(NCORES):
        in_maps.append({
            "hiddenT": hT,
            "targetT": tT,
            "cosT": cT,
            "sinT": sT,
            "wqT": np.ascontiguousarray(Wq[512 * c:512 * c + 512, :].T).astype(bf16),
            "wkT": np.ascontiguousarray(Wk[128 * c:128 * c + 128, :].T).astype(bf16),
            "wvT": np.ascontiguousarray(Wv[128 * c:128 * c + 128, :].T).astype(bf16),
            "woT": np.ascontiguousarray(Wo[:, 512 * c:512 * c + 512].T).astype(bf16),
            "ones_bf": np.ones((128, 1), dtype=bf16),
            "onesr": onesr,
            "ident": ident,
        })
    return in_maps


_CACHE = {}
LAST_EXEC_NS = None
TRACE = False


def kernel(hidden_states, target_hidden, cos, sin, Wq, Wk, Wv, Wo):
    global LAST_EXEC_NS
    if "nc" not in _CACHE:
        _CACHE["nc"] = build_program()
    nc = _CACHE["nc"]
    in_maps = make_in_maps(
        hidden_states, target_hidden, cos, sin, Wq, Wk, Wv, Wo
    )
    res = run_bass_kernel_spmd(
        nc, in_maps, list(range(NCORES)), trace=TRACE
    )
    LAST_EXEC_NS = res.exec_time_ns
    out = np.zeros((QL, H), dtype=np.float32)
    for r in res.results:
        out += r["out"]
    return out.reshape(1, QL, H)
